# revision 15
# baseline (speedup 1.0000x reference)
"""v6: per-row analytic tau0 + Newton counts; all heavy ops in DVE 4x /
2x perf modes, counts split DVE||ACT, tgt reduced to single-src top-K.

Key facts this build exploits (measured):
  - DVE tensor_scalar fp16 single-src runs 4x (4.5us / 16K cols); with
    accum_out it drops to 1x, so monolithic fused counts cost 17.3us -
    but per-2048-tile fused counts are 2.29us and can run 4 tiles on
    DVE while ACT Sign(+accum) handles the other 4 in parallel (~9us
    wall per count).
  - ACT activation is ~1.89us/2048-tile any dtype; Sign with bias=-tau
    emits sign(P-tau); accum_out gives its sum (count = (FD+sum)/2);
    with uint8 output Sign saturates {-1,0,1} -> {0,1} = an is_ge mask.
  - scalar_tensor_tensor and tensor_reduce are 1x on DVE: avoided.

Algorithm: P16 = fp16(Ln(U0 * SC')) where SC' = sqrt(Ut)*prefix*
exp(-tau0_row) folds the slot constant and the host-analytic threshold
(from Ut only) into the Ln scale, so thresholds start at 0 where fp16
ulp is tiny.  src: N1 counts ride the load (DVE fused tiles), Newton,
N2 (DVE fused mono), Newton -> tau2; mask = ACT Sign->u8.  tgt: R16 =
P16t + 1024*(P16s < tau2) pushes non-src tokens far above any
threshold, making #{R16>=th} EXACTLY the penalized count and the tgt
mask a single is_ge -> the same single-src top-K as src.
"""

import sys
import functools
import numpy as np

sys.path.insert(0, "/opt/trn_rl_repo")

B, N, T = 128, 131072, 64
HW = N // T
N_CORES = 8
RPC = B // N_CORES          # rows per core
PPR = 128 // RPC            # partitions per row
FD = N // PPR               # free dim per partition
NT = FD // HW               # slots per partition
EPS = 1e-3
LOG1E9 = float(np.log(np.float32(1e-9)))
TW = 2048                   # tile width
NTILES = FD // TW
ND = 5                      # count tiles on DVE (fused); rest on ACT (Sign)
NDF = 5                     # final-mask tiles on DVE; rest on ACT
TGT_N2 = False              # second tgt Newton count (slot-corr makes it optional)
BIG = 1024.0


# ---------------- host analytics (Ut + K only) ----------------

def _surv(x):
    return np.where(x <= EPS, 1.0, np.where(x > 1 - EPS, 0.0, 1.0 - x))


def _solve_tau(c, K, lo, hi, iters=70):
    lo = np.full(c.shape[0], lo)
    hi = np.full(c.shape[0], hi)
    for _ in range(iters):
        mid = 0.5 * (lo + hi)
        cnt = (HW * _surv(np.exp(mid[:, None] - c))).sum(1)
        hi = np.where(cnt > K, hi, mid)
        lo = np.where(cnt > K, mid, lo)
    return 0.5 * (lo + hi)


def _host_analytics(Ut_src, Ut_tgt, K_src, K_tgt):
    L = np.linspace(1.0, 0.001, T, dtype=np.float32) ** np.float32(1.0 / 3.0)
    cs = np.log(Ut_src.astype(np.float64)) / 2 + np.log(L.astype(np.float64))[None]
    ct = np.log(Ut_tgt.astype(np.float64)) / 2
    tau0_s = _solve_tau(cs, K_src, -15.0, 0.0)
    x = np.exp(tau0_s[:, None] - cs)
    act = (x > EPS) & (x <= 1 - EPS)
    inv_s = 1.0 / (HW * x * act).sum(1)
    ms = HW * _surv(x)                       # expected src tokens per slot
    assert K_tgt > N - K_src + 4000, "needs tgt threshold in penalized zone"
    lo = np.full(B, -35.0)
    hi = np.full(B, 0.0)
    for _ in range(70):
        mid = 0.5 * (lo + hi)
        cnt = ((HW - ms) * _surv(np.exp(mid[:, None] - ct))
               + ms * _surv(np.exp(mid[:, None] - LOG1E9 - ct))).sum(1)
        hi = np.where(cnt > K_tgt, hi, mid)
        lo = np.where(cnt > K_tgt, mid, lo)
    tau0_t = 0.5 * (lo + hi)
    q0 = tau0_t - LOG1E9                      # base-space center
    xt = np.exp(q0[:, None] - ct)
    actt = (xt > EPS) & (xt <= 1 - EPS)
    inv_t = 1.0 / (ms * xt * actt).sum(1)
    SCs = np.exp(cs - tau0_s[:, None]).astype(np.float32)   # [B,T]
    SCt = np.exp(ct - q0[:, None]).astype(np.float32)       # [B,T]
    # slot-correction consts for the tgt theta0: predicted (cnt - K_tgt) =
    # sum_s (n_s - ms)*W_s + tau2*C with W = x_t - 1, C = sum slope_s*(1-x_t)
    x_t = _surv(xt)
    W = (x_t - 1.0)
    slope_s = HW * x * act
    C = (slope_s * (1.0 - x_t)).sum(1)
    DC = (ms * W).sum(1)
    return (SCs, SCt, inv_s.astype(np.float32), inv_t.astype(np.float32),
            W.astype(np.float32), C.astype(np.float32), DC.astype(np.float32))


def _per_core_consts(SCs, SCt, inv_s, inv_t, W, C, DC, core):
    rs = slice(core * RPC, (core + 1) * RPC)
    scs_c, sct_c, w_c = SCs[rs], SCt[rs], W[rs]
    invs_c, invt_c, c_c, dc_c = inv_s[rs], inv_t[rs], C[rs], DC[rs]
    # packed const block: [scp_s | scp_t | w | ivs | ivt | c | dc | gm]
    cb = np.zeros((128, 3 * NT + 4 + 128), dtype=np.float32)
    for p in range(128):
        r, jp = p // PPR, p % PPR
        cb[p, 0:NT] = scs_c[r, jp * NT:(jp + 1) * NT]
        cb[p, NT:2 * NT] = sct_c[r, jp * NT:(jp + 1) * NT]
        cb[p, 2 * NT:3 * NT] = w_c[r, jp * NT:(jp + 1) * NT]
        cb[p, 3 * NT] = invs_c[r]
        cb[p, 3 * NT + 1] = invt_c[r]
        cb[p, 3 * NT + 2] = c_c[r]
        cb[p, 3 * NT + 3] = dc_c[r] / PPR  # rowsum of DC-col gives DC back
        g = p // PPR
        cb[p, 3 * NT + 4 + g * PPR:3 * NT + 4 + (g + 1) * PPR] = 1.0
    return cb


# ---------------- device kernel ----------------

@functools.lru_cache(maxsize=4)
def _build(k_src: int, k_tgt: int):
    import concourse.bass as bass
    import concourse.tile as tile
    from concourse import bacc, mybir
    from concourse.alu_op_type import AluOpType as op
    from contextlib import ExitStack

    f32 = mybir.dt.float32
    f16 = mybir.dt.float16
    u8 = mybir.dt.uint8
    AF = mybir.ActivationFunctionType

    nc = bacc.Bacc("TRN2", target_bir_lowering=False, debug=False,
                   num_devices=N_CORES)

    NCB = 3 * NT + 4 + 128
    u0s = nc.dram_tensor("u0s", [RPC, N], f32, kind="ExternalInput")
    u0t = nc.dram_tensor("u0t", [RPC, N], f32, kind="ExternalInput")
    cb_d = nc.dram_tensor("cb", [128, NCB], f32, kind="ExternalInput")
    ms_d = nc.dram_tensor("ms", [RPC, N], u8, kind="ExternalOutput")
    mt_d = nc.dram_tensor("mt", [RPC, N], u8, kind="ExternalOutput")

    # ACT-side count tiles cover this many elements per row (for the
    # sign-sum -> count conversion)
    FD_ACT_ROW = (NTILES - ND) * TW * PPR

    with tile.TileContext(nc) as tc, ExitStack() as ctx:
        pool = ctx.enter_context(tc.tile_pool(name="big", bufs=1))
        stage = ctx.enter_context(tc.tile_pool(name="stage", bufs=4))
        outp = ctx.enter_context(tc.tile_pool(name="outp", bufs=4))
        psum = ctx.enter_context(tc.tile_pool(name="ps", bufs=2, space="PSUM"))

        P16s = pool.tile([128, FD], f16, tag="P16s")
        P16t = pool.tile([128, FD], f16, tag="P16t")
        R16 = pool.tile([128, FD], f16, tag="R16")
        JNK = pool.tile([128, FD], f16, tag="JNK")   # count outs + 1024*(1-m)
        M8 = pool.tile([128, FD], u8, tag="M8")      # src mask u8
        CB = pool.tile([128, NCB], f32, tag="CB")
        SCPS = CB[:, 0:NT]
        SCPT = CB[:, NT:2 * NT]
        WSL = CB[:, 2 * NT:3 * NT]
        IVS = CB[:, 3 * NT:3 * NT + 1]
        IVT = CB[:, 3 * NT + 1:3 * NT + 2]
        CSL = CB[:, 3 * NT + 2:3 * NT + 3]
        DCSL = CB[:, 3 * NT + 3:3 * NT + 4]
        GM = CB[:, 3 * NT + 4:3 * NT + 4 + 128]
        CNT8 = pool.tile([128, NTILES], f32, tag="CNT8")
        CNTN1 = pool.tile([128, NTILES], f32, tag="CNTN1")
        T1 = pool.tile([128, NTILES], f32, tag="T1")
        CA = pool.tile([128, 1], f32, tag="CA")
        CBS = pool.tile([128, 1], f32, tag="CBS")
        D1 = pool.tile([128, 1], f32, tag="D1")
        TAU = pool.tile([128, 1], f32, tag="TAU")
        NTAU = pool.tile([128, 1], f32, tag="NTAU")
        TH = pool.tile([128, 1], f32, tag="TH")
        NTH = pool.tile([128, 1], f32, tag="NTH")

        nc.sync.dma_start(CB[:], cb_d.ap())
        nc.vector.memset(TAU[:], 0.0)
        nc.vector.memset(TH[:], 0.0)
        # dummy: force the Ln/Sign ACT table load before any data arrives
        nc.scalar.activation(NTH[:], TAU[:], AF.Ln, scale=1.0)

        u0s_r = u0s.ap().rearrange("r (jp f) -> (r jp) f", jp=PPR)
        u0t_r = u0t.ap().rearrange("r (jp f) -> (r jp) f", jp=PPR)
        ms_r = ms_d.ap().rearrange("r (jp f) -> (r jp) f", jp=PPR)
        mt_r = mt_d.ap().rearrange("r (jp f) -> (r jp) f", jp=PPR)

        # ---- src load; N1 fused counts on DVE ride the DMA
        with nc.named_scope("load_src"):
            for j in range(NTILES):
                sl = slice(j * TW, (j + 1) * TW)
                st = stage.tile([128, TW], f32, tag="stg")
                nc.sync.dma_start(st[:], u0s_r[:, sl])
                nc.scalar.activation(P16s[:, sl], st[:], AF.Ln,
                                     scale=SCPS[:, j:j + 1])
                nc.vector.tensor_scalar(JNK[:, sl], P16s[:, sl], 0.0, None,
                                        op0=op.is_ge, op1=op.add,
                                        accum_out=CNTN1[:, j:j + 1])

        # ---- tgt load
        with nc.named_scope("load_tgt"):
            for j in range(NTILES):
                sl = slice(j * TW, (j + 1) * TW)
                st = stage.tile([128, TW], f32, tag="stg")
                nc.sync.dma_start(st[:], u0t_r[:, sl])
                nc.scalar.activation(P16t[:, sl], st[:], AF.Ln,
                                     scale=SCPT[:, j:j + 1])

        def newton_full(tau_ap, k_f, inv_ap):
            """counts from all NTILES cols of CNTN1 (DVE fused N1 counts)."""
            ps = psum.tile([128, NTILES], f32, tag="psA")
            nc.tensor.matmul(ps[:], GM, CNTN1[:, 0:NTILES], start=True,
                             stop=True)
            nc.vector.tensor_reduce(CA[:], ps[:], axis=mybir.AxisListType.X,
                                    op=op.add)
            nc.vector.tensor_scalar(D1[:], CA[:], k_f, None, op0=op.subtract)
            nc.vector.tensor_mul(D1[:], D1[:], inv_ap)
            nc.vector.tensor_add(tau_ap, tau_ap, D1[:])

        def newton_split(tau_ap, k_f, inv_ap):
            """cols 0:ND = DVE raw counts, ND: = ACT sign sums.
            cnt = A + B/2 + FD_ACT_ROW/2  ->  tau += (cnt-K)*inv"""
            ps = psum.tile([128, NTILES], f32, tag="psA")
            nc.tensor.matmul(ps[:], GM, CNT8[:, 0:NTILES], start=True,
                             stop=True)
            nc.vector.tensor_reduce(CA[:], ps[:, 0:ND], axis=mybir.AxisListType.X,
                                    op=op.add)
            nc.vector.tensor_reduce(CBS[:], ps[:, ND:NTILES],
                                    axis=mybir.AxisListType.X, op=op.add)
            nc.vector.tensor_scalar(CBS[:], CBS[:], 0.5, FD_ACT_ROW / 2.0 - k_f,
                                    op0=op.mult, op1=op.add)
            nc.vector.tensor_add(D1[:], CA[:], CBS[:])
            nc.vector.tensor_mul(D1[:], D1[:], inv_ap)
            nc.vector.tensor_add(tau_ap, tau_ap, D1[:])

        THZ = pool.tile([128, 1], f32, tag="THZ")
        NTAU2 = pool.tile([128, 1], f32, tag="NTAU2")
        TAU2 = pool.tile([128, 1], f32, tag="TAU2")
        ZD = pool.tile([128, 1], f32, tag="ZD")

        def count_split(src_tile, thr_ap, nthr_ap):
            """DVE fused on tiles [0,ND), ACT Sign on [ND,NTILES)."""
            for j in range(ND):
                sl = slice(j * TW, (j + 1) * TW)
                nc.vector.tensor_scalar(JNK[:, sl], src_tile[:, sl], thr_ap,
                                        None, op0=op.is_ge, op1=op.add,
                                        accum_out=CNT8[:, j:j + 1])
            for j in range(ND, NTILES):
                sl = slice(j * TW, (j + 1) * TW)
                nc.scalar.activation(JNK[:, sl], src_tile[:, sl], AF.Sign,
                                     bias=nthr_ap,
                                     accum_out=CNT8[:, j:j + 1])

        # ---- src Newton: one step -> tau1; src mask and tgt cond both at tau1
        with nc.named_scope("topk_src"):
            newton_full(TAU[:], float(k_src), IVS)           # -> tau1
            nc.vector.tensor_scalar(NTAU[:], TAU[:], -1.0, None, op0=op.mult)

        # ---- JNK = 1024*(P16s < tau1); R16 tiles chase the tgt Ln tiles
        # (all hidden under the tgt load window)
        with nc.named_scope("build_r16"):
            nc.vector.tensor_scalar(JNK[:], P16s[:], TAU[:], BIG,
                                    op0=op.is_lt, op1=op.mult)
            for j in range(NTILES):
                sl = slice(j * TW, (j + 1) * TW)
                nc.vector.tensor_add(R16[:, sl], P16t[:, sl], JNK[:, sl])

        # ---- tgt: one split count at theta0=0, Newton -> th1, mask at th1
        with nc.named_scope("topk_tgt"):
            # THZ = 0, but reads the last P16t column: forces the ACT Sign
            # tiles (bias=THZ) to sit after the tgt Ln tiles in the queue
            nc.scalar.activation(THZ[:], P16t[:, FD - 1:FD], AF.Identity,
                                 scale=0.0)
            count_split(R16, 0.0, THZ[:])                    # T-N1 at 0
            # TAU2 = tau1, dep-chained to the last DVE count accum: the
            # 3 DVE src-mask tiles fill the newton/sign-wait shadow
            nc.vector.scalar_tensor_tensor(TAU2[:], CNT8[:, ND - 1:ND], 0.0,
                                           TAU[:], op0=op.mult, op1=op.add)
            for j in range(3):
                sl = slice(j * TW, (j + 1) * TW)
                nc.vector.tensor_scalar(M8[:, sl], P16s[:, sl], TAU2[:], None,
                                        op0=op.is_ge)
            newton_split(TH[:], float(k_tgt), IVT)           # -> th1
            # final tgt mask: all-DVE u8 tiles with DMA chasing
            for j in range(NTILES):
                sl = slice(j * TW, (j + 1) * TW)
                ot = outp.tile([128, TW], u8, tag="ot")
                nc.vector.tensor_scalar(ot[:], R16[:, sl], TH[:], None,
                                        op0=op.is_ge)
                nc.sync.dma_start(mt_r[:, sl], ot[:])

        # ---- src mask at tau1: 5 tiles on ACT (Sign u8, after the T-N1
        # signs via the NTAU2 chain); tiles 0-2 were made on DVE above
        with nc.named_scope("mask_src"):
            nc.scalar.activation(NTAU2[:], CNT8[:, NTILES - 1:NTILES],
                                 AF.Identity, scale=0.0, bias=NTAU[:])
            for j in range(3, NTILES):
                sl = slice(j * TW, (j + 1) * TW)
                nc.scalar.activation(M8[:, sl], P16s[:, sl], AF.Sign,
                                     bias=NTAU2[:])
                nc.sync.dma_start(ms_r[:, sl], M8[:, sl])
            for j in range(3):
                sl = slice(j * TW, (j + 1) * TW)
                nc.sync.dma_start(ms_r[:, sl], M8[:, sl])

    nc.compile()
    return nc


def _in_maps(U0_src, Ut_src, U0_tgt, Ut_tgt, K_src, K_tgt):
    SCs, SCt, inv_s, inv_t, W, C, DC = _host_analytics(Ut_src, Ut_tgt, K_src, K_tgt)
    maps = []
    for c in range(N_CORES):
        cb = _per_core_consts(SCs, SCt, inv_s, inv_t, W, C, DC, c)
        rs = slice(c * RPC, (c + 1) * RPC)
        maps.append({
            "u0s": np.ascontiguousarray(U0_src[rs]),
            "u0t": np.ascontiguousarray(U0_tgt[rs]),
            "cb": cb,
        })
    return maps


def run(U0_src, Ut_src, U0_tgt, Ut_tgt, K_src, K_tgt, trace=False,
        trace_kwargs=None):
    import time
    from concourse.bass_utils import run_bass_kernel_spmd
    nc = _build(int(K_src), int(K_tgt))
    maps = _in_maps(np.asarray(U0_src, np.float32), np.asarray(Ut_src, np.float32),
                    np.asarray(U0_tgt, np.float32), np.asarray(Ut_tgt, np.float32),
                    int(K_src), int(K_tgt))
    try:
        res = run_bass_kernel_spmd(nc, maps, list(range(N_CORES)), trace=trace,
                                   **(trace_kwargs or {}))
    except Exception:
        # transient NRT exec-unit failures have been observed; retry once
        time.sleep(15)
        res = run_bass_kernel_spmd(nc, maps, list(range(N_CORES)), trace=trace,
                                   **(trace_kwargs or {}))
    src = np.concatenate([res.results[c]["ms"] for c in range(N_CORES)], axis=0)
    tgt = np.concatenate([res.results[c]["mt"] for c in range(N_CORES)], axis=0)
    return (src != 0, tgt != 0), res


def kernel(U0_src, Ut_src, U0_tgt, Ut_tgt, K_src, K_tgt):
    (src, tgt), _ = run(U0_src, Ut_src, U0_tgt, Ut_tgt, K_src, K_tgt)
    return (src, tgt)
